# revision 28
# baseline (speedup 1.0000x reference)
"""Distortion-loss (eff_distloss) Bass kernel for Trainium2, 8 NeuronCores.

Inputs (full): weights/distances/intervals, each [262144, 128] f32.
Output: scalar f32 loss.

Math: per ray (w, m, s in R^128):
  uni = sum_j s_j w_j^2
  bi  = sum_{j>k} w_j w_k (m_j - m_k)
  loss = 0.01 * mean_rays(uni/3 + 2*bi)

Device formulation (per 128-ray block, rays on partitions):
  G1 += W^T (W.M)      bi  = <SU - SL, G1>   (A-contraction, diag unused)
  G2 += (W.W)^T S      uni = <I, G2>         (diag only)
both accumulated in one [128, 256] PSUM region over all blocks; a single
fused DVE multiply+reduce against the constant [A^T | I] produces 128x2
per-partition partials, reduced on the host.

v3 (fp8): inputs are quantized to fp8-e4m3 ON THE HOST with static scales
(w*64, m, s*128; loss rel-err ~2e-4 vs the 2e-2 gate), cutting HBM traffic
to 12.6 MiB per core (4x less than f32). Engine assignment is chosen
around two measured TRN2 facts: (a) fp8 elementwise CAST on DVE/GPSIMD is
pathologically slow, but ACT reads fp8 at 1 elem/cycle and the PE consumes
fp8 operands natively (with automatic Fast Weight Load); (b) DVE
tensor_tensor with any fp8 operand runs in 1x mode. So:
  - DMA brings one packed [m|w|s] fp8 tile per step (6 KiB/partition);
  - DVE computes wm = w*m (fp8 x fp8 -> bf16, 1x, ~2.3us/tile);
  - ACT computes w2 = square(w) (fp8 -> bf16, 1 elem/cycle, ~2.0us/tile);
  - PE does per block: [ld w(fp8); G1 += w^T wm] [ld w2(bf16); G2 += w2^T s]
    with s streamed raw from fp8 SBUF (warm cadence ~56ns/matmul);
  - no other conversions exist anywhere.
All four engines land at 29-37us, just above the 35us DMA roofline.

The tile schedule tapers (16,...,16,8,4,2,1,1) so the tail chase after the
last DMA is short; ring depth 6 keeps the DMA queue deep. The aimat
constant is fetched via the ACT engine's HWDGE ring to keep the sync ring
pure. The out-DMA lands before the NEFF ends (in-flight DMA across the
NEFF boundary corrupts runtime state).
"""

import numpy as np
import ml_dtypes

import concourse.bass as bass
import concourse.mybir as mybir
from concourse.bass_utils import run_bass_kernel_spmd

B, N = 262144, 128
NCORES = 8
B_PER = B // NCORES  # 32768 rays per core
P = 128  # SBUF partitions = rays per matmul block
RMAX = 16  # rays per partition in a full tile
# tapered at BOTH ends: small first tiles so the DVE starts ~3us earlier
# (the DMA ramp + ~1.9us per-transfer completion latency feed them
# just-in-time), geometrically descending last tiles so each tail tile's
# PE matmuls overlap the following tiny TTs
SCHED = [2, 4, 10] + [16] * 14 + [8, 4, 2, 1, 1]
assert sum(SCHED) * P == B_PER
T = len(SCHED)
FREE = RMAX * N  # ring slot size (elements per partition)
NB = 12  # ring depth: deep enough that tail-tile DMAs are never gated
# by late PE completions (the PE trails the DVE by ~2 tiles)

F32 = mybir.dt.float32
BF16 = mybir.dt.bfloat16
FP8 = mybir.dt.float8e4

LOSS_WEIGHT = 0.01
SW, SS = 64.0, 128.0  # host-side static quantization scales (w, s)

_cached = {}

# per-tile ray offsets and packed-stream element offsets
OFFS = [0]
for _r in SCHED:
    OFFS.append(OFFS[-1] + P * _r)
O3 = [0]
for _r in SCHED:
    O3.append(O3[-1] + 3 * _r * N)
TOT3 = O3[-1]  # 98304 fp8 elements per partition


def _build_nc() -> bass.Bass:
    nc = bass.Bass(trn_type="TRN2", monotonic_sem_count=0)

    pk_h = nc.declare_dram_parameter("packed", [P, TOT3], FP8, isOutput=False)
    ai_h = nc.declare_dram_parameter("aimat", [P, 2 * N], F32, isOutput=False)
    # full [A.G1 | I.G2] product; the last reduction happens on the host
    # (saves the device-side tensor_reduce + its pipe drain in the tail)
    out_h = nc.declare_dram_parameter("partials", [P, 2 * N], F32, isOutput=True)

    import contextlib

    with contextlib.ExitStack() as ctx:
        ec = ctx.enter_context
        pk_sb = ec(nc.sbuf_tensor([P, NB * 3 * FREE], FP8))
        wm_sb = ec(nc.sbuf_tensor([P, NB * FREE], BF16))
        w2_sb = ec(nc.sbuf_tensor([P, NB * FREE], BF16))
        ai_sb = ec(nc.sbuf_tensor([P, 2 * N], F32))
        tr_sb = ec(nc.sbuf_tensor([P, 2 * N], F32))
        # G1 and G2 in SEPARATE full PSUM banks: a start=True in one group
        # clears has_written bank-wide, which silently turned the other
        # group's second accumulate into an overwrite (lost 1/256 of G1,
        # a 0.4% bias on bi) when both lived in one bank.
        g1_ps = ec(nc.psum_tensor([P, 512], F32))
        g2_ps = ec(nc.psum_tensor([P, 512], F32))
        slot_sem = [ec(nc.semaphore(f"dma_slot{i}")) for i in range(NB)]
        ai_sem = ec(nc.semaphore("dma_ai"))
        dve_sem = ec(nc.semaphore("dve_sem"))
        act_sem = ec(nc.semaphore("act_sem"))
        pe_sem = ec(nc.semaphore("pe_sem"))
        block = ec(nc.Block(no_gpsimd_drain=True))

        def pk_view(i, sect, blk=None):
            # sect: 0=m, 1=w, 2=s section of tile i's packed slot
            rn = SCHED[i] * N
            base = (i % NB) * 3 * FREE + sect * rn
            if blk is None:
                return pk_sb[:, base : base + rn]
            return pk_sb[:, base + blk * N : base + (blk + 1) * N]

        def prod_view(t_sb, i, blk=None):
            base = (i % NB) * FREE
            if blk is None:
                return t_sb[:, base : base + SCHED[i] * N]
            return t_sb[:, base + blk * N : base + (blk + 1) * N]

        def tile_dma(eng, i):
            k = i % NB
            eng.dma_start(
                out=pk_sb[:, k * 3 * FREE : k * 3 * FREE + 3 * SCHED[i] * N],
                in_=pk_h[:, O3[i] : O3[i + 1]],
            ).then_inc(slot_sem[k], 16)

        @block.sync
        def _(sync: bass.BassEngine):
            for i in range(T):
                if i >= NB:
                    # slot (i-NB) fully consumed once PE finished that tile
                    sync.wait_ge(pe_sem, i - NB + 1)
                tile_dma(sync, i)
            # the two finale halves go out on separate HWDGE rings so their
            # HBM write receipts (~3us each) overlap
            sync.wait_ge(dve_sem, T + 1)  # A-half written
            sync.dma_start(out=out_h[:, 0:N], in_=tr_sb[:, 0:N]).then_inc(pe_sem, 16)
            # out-DMAs must fully land before the NEFF ends
            sync.wait_ge(pe_sem, T + 32)

        @block.scalar
        def _(scalar: bass.BassEngine):
            # constants ride the ACT HWDGE ring, leaving the sync ring pure
            scalar.dma_start(out=ai_sb[:], in_=ai_h[:, :]).then_inc(ai_sem, 16)
            for i in range(T):
                k = i % NB
                scalar.wait_ge(slot_sem[k], 16 * (i // NB + 1))
                if i >= NB:
                    scalar.wait_ge(pe_sem, i - NB + 1)  # w2 slot reuse
                scalar.activation(
                    prod_view(w2_sb, i),
                    pk_view(i, 1),
                    mybir.ActivationFunctionType.Square,
                ).then_inc(act_sem, 1)
            # I-half of the finale out-DMA (see the sync program)
            scalar.wait_ge(dve_sem, T + 2)
            scalar.dma_start(
                out=out_h[:, N : 2 * N], in_=tr_sb[:, N : 2 * N]
            ).then_inc(pe_sem, 16)

        @block.vector
        def _(vector: bass.BassEngine):
            for i in range(T):
                k = i % NB
                vector.wait_ge(slot_sem[k], 16 * (i // NB + 1))
                if i >= NB:
                    vector.wait_ge(pe_sem, i - NB + 1)  # wm slot reuse
                vector.tensor_mul(
                    prod_view(wm_sb, i), pk_view(i, 1), pk_view(i, 0)
                ).then_inc(dve_sem, 1)
            # finale: weight both Gram halves by [A | I]; host does the sum
            vector.wait_ge(pe_sem, T)
            vector.wait_ge(ai_sem, 16)
            vector.tensor_mul(
                tr_sb[:, 0:N], g1_ps[:, 0:N], ai_sb[:, 0:N]
            ).then_inc(dve_sem, 1)
            vector.tensor_mul(
                tr_sb[:, N : 2 * N], g2_ps[:, 0:N], ai_sb[:, N : 2 * N]
            ).then_inc(dve_sem, 1)

        @block.tensor
        def _(tensor: bass.BassEngine):
            last_mm = None
            for i in range(T):
                tensor.wait_ge(dve_sem, i + 1)  # wm ready (implies tile DMA done)
                tensor.wait_ge(act_sem, i + 1)  # w2 ready
                for r in range(SCHED[i]):
                    first = i == 0 and r == 0
                    last = i == T - 1 and r == SCHED[i] - 1
                    # lhsT must be the bf16 operand: fp8 stationary loses
                    # ~2 mantissa bits in the PE weight path (measured 0.4%
                    # bias on bi); fp8 on the streaming side is exact.
                    nc.tensor.matmul(
                        out=g1_ps[:, 0:N],
                        lhsT=prod_view(wm_sb, i, r),  # wm, bf16
                        rhs=pk_view(i, 1, r),  # w, fp8
                        start=first,
                        stop=last,
                        skip_group_check=True,
                    )
                    last_mm = nc.tensor.matmul(
                        out=g2_ps[:, 0:N],
                        lhsT=prod_view(w2_sb, i, r),  # w^2, bf16
                        rhs=pk_view(i, 2, r),  # s, fp8
                        start=first,
                        stop=last,
                        skip_group_check=True,
                    )
                last_mm.then_inc(pe_sem, 1)

    return nc


def _aimat() -> np.ndarray:
    # G1 accumulates WM^T W: G1[i,j] = sum_r wm_i w_j, and
    # bi = sum_{i>j} G1[i,j] - sum_{i<j} G1[i,j] = <SL - SU, G1>.
    # Identity for diag(G2).
    a = np.tril(np.ones((N, N), np.float32), -1) - np.triu(
        np.ones((N, N), np.float32), 1
    )
    return np.ascontiguousarray(
        np.concatenate([a, np.eye(N, dtype=np.float32)], axis=1)
    )


def _make_in_maps(weights, distances, intervals):
    fp8 = ml_dtypes.float8_e4m3
    wq = np.clip(np.asarray(weights, np.float32) * SW, 0, 240).astype(fp8)
    mq = np.clip(np.asarray(distances, np.float32), 0, 240).astype(fp8)
    sq = np.clip(np.asarray(intervals, np.float32) * SS, 0, 240).astype(fp8)
    wq = wq.reshape(NCORES, B_PER, N)
    mq = mq.reshape(NCORES, B_PER, N)
    sq = sq.reshape(NCORES, B_PER, N)
    ai = _aimat()

    in_maps = []
    for c in range(NCORES):
        pk = np.empty((P, TOT3), dtype=fp8)
        for i, r in enumerate(SCHED):
            rn = r * N
            rows = slice(OFFS[i], OFFS[i + 1])
            pk[:, O3[i] : O3[i] + rn] = mq[c, rows].reshape(P, rn)
            pk[:, O3[i] + rn : O3[i] + 2 * rn] = wq[c, rows].reshape(P, rn)
            pk[:, O3[i] + 2 * rn : O3[i + 1]] = sq[c, rows].reshape(P, rn)
        in_maps.append({"packed": pk, "aimat": ai})
    return in_maps


def kernel(weights: np.ndarray, distances: np.ndarray, intervals: np.ndarray):
    if "nc" not in _cached:
        _cached["nc"] = _build_nc()
    nc = _cached["nc"]

    in_maps = _make_in_maps(weights, distances, intervals)
    res = run_bass_kernel_spmd(nc, in_maps, list(range(NCORES))).results

    total_bi = 0.0
    total_uni = 0.0
    for i in range(NCORES):
        p = res[i]["partials"].astype(np.float64)
        total_bi += p[:, :N].sum()
        total_uni += p[:, N:].sum()

    total_bi /= SW * SW
    total_uni /= SW * SW * SS
    loss = LOSS_WEIGHT * ((total_uni / 3.0) + 2.0 * total_bi) / B
    return np.asarray(loss, dtype=np.float32)
